# revision 2
# baseline (speedup 1.0000x reference)
"""Weighted-Dice-loss (nn_DiceLoss) Trainium2 Bass kernel, v5 (scan-free).

Full inputs: pred [64,1,512,512] f32, mask [64,1,512,512] f32.
Output: scalar f32 = mean over images of 1 - (2*inter+0.5)/(union+0.5) with
  weit  = 1 + 5*|boxavg31(mask) - mask|
  inter = sum(sigmoid(pred)*mask*weit),  union = sum((sigmoid(pred)+mask)*weit)

Data-parallel: 8 images per core.  All device tensors live in TRANSPOSED
layout [w-partitions, h-free]; host pre-transposes and casts to bf16
(8.4 MB/core HBM vs 16.8 f32 normal).

Per image (PB=128 partitions; w-blocks k/W, h-blocks j in 0..3):
  PE W-pass   u[h,w'] = sum_w M[h,w]*BW[w,w'] : stationary = maskT block
              [w-in-k, h-in-j], moving = band window (158 cols from one
              constant [128,512] band tile).  k=1 runs full-width 512 with
              start=True to zero the PSUM; k=0/2/3 add 143-158-col windows.
              -> U_j [h-part, w-free] PSUM, 16 matmuls, ~956 cols/j.
  u-copy      u' = (5/961)*U_j -> SBUF bf16 (ACT scale-copy / DVE
              tensor_scalar mult; engine alternates by image for balance).
  PE H-pass   D^T[w,h'] = sum_h u'[h,w]*BH[h,h'] - 5*M^T[w,h'] : per w-block
              W: ident first (stationary=-5*I exact bf16, moving=maskT_W,
              512 cols, start=True zeroes) then 4 j-windows with
              stationary = u'_j blocks.  D = 5*(boxavg - mask).
  ACT         a_W = Abs(D_W) -> bf16  (= weit-1);  p = Sigmoid(predT)
              with accum_out -> sum(p)
  DVE         s2 = (a+1)*maskT  (STT, accum -> sum(m*weit))
              pa = p*a   (TT 2x);   tt = s2*p  (TT 2x)
  PE reds     ones-matmuls of pa/tt k-blocks -> per-image rows (partition
              offset 32*(b%4)) of persistent PSUM red tiles, accumulated
              across all 8 images; single bulk PSUM->SBUF copy at the end.
Host: per-image inter/union from acc columns + red rows, wdiss, mean.
"""

import numpy as np
import ml_dtypes
from contextlib import ExitStack

import concourse.tile as tile
from concourse import bacc, mybir
from concourse.bass_utils import run_bass_kernel_spmd

N_CORES = 8
B_PER_CORE = 8
H = W = 512
PB = 128
NB = 4                    # 128-blocks per axis
f32 = mybir.dt.float32
bf16 = mybir.dt.bfloat16
Alu = mybir.AluOpType
Act = mybir.ActivationFunctionType

# band windows: Bsm[r, s] = 1 iff 113+r <= s <= 143+r  (s in 0..511)
# block-local slices (see derivation in session notes):
#   full-width (k=1):          Bsm[:, 0:512]
#   first block (k/j = 0):     Bsm[:, 128:271] -> out cols [0,143)
#   interior:                  Bsm[:, 113:271] -> out cols [128x-15, 128x+143)
#   last block (k/j = 3):      Bsm[:, 113:256] -> out cols [369,512)
WIN = {0: (128, 271, 0, 143), 2: (113, 271, 241, 399), 3: (113, 256, 369, 512)}
HWIN = {0: (128, 271, 0, 143), 1: (113, 271, 113, 271),
        2: (113, 271, 241, 399), 3: (113, 256, 369, 512)}


def _host_constants():
    r = np.arange(PB)[:, None]
    s = np.arange(W)[None, :]
    bsm = ((113 + r <= s) & (s <= 143 + r)).astype(ml_dtypes.bfloat16)
    negi5 = (-5.0 * np.eye(PB)).astype(ml_dtypes.bfloat16)
    return bsm, negi5


def _build():
    nc = bacc.Bacc("TRN2", target_bir_lowering=False, debug=False,
                   num_devices=N_CORES)
    maskT_d = nc.dram_tensor("maskT", [B_PER_CORE, PB, NB * W], bf16,
                             kind="ExternalInput")
    predT_d = nc.dram_tensor("predT", [B_PER_CORE, PB, NB * W], bf16,
                             kind="ExternalInput")
    bsm_d = nc.dram_tensor("bsm", [PB, W], bf16, kind="ExternalInput")
    negi5_d = nc.dram_tensor("negi5", [PB, PB], bf16, kind="ExternalInput")
    acc_d = nc.dram_tensor("acc", [PB, 24], f32, kind="ExternalOutput")
    red_d = nc.dram_tensor("red", [PB, 3, W], f32, kind="ExternalOutput")

    with tile.TileContext(nc) as tc:
        with ExitStack() as ctx:
            cpool = ctx.enter_context(tc.tile_pool(name="cpool", bufs=1))
            mpool = ctx.enter_context(tc.tile_pool(name="mpool", bufs=3))
            ppool = ctx.enter_context(tc.tile_pool(name="ppool", bufs=3))
            upool = ctx.enter_context(tc.tile_pool(name="upool", bufs=3))
            apool = ctx.enter_context(tc.tile_pool(name="apool", bufs=2))
            sgpool = ctx.enter_context(tc.tile_pool(name="sgpool", bufs=3))
            scr = ctx.enter_context(tc.tile_pool(name="scr", bufs=3))
            upsum = ctx.enter_context(
                tc.tile_pool(name="upsum", bufs=3, space="PSUM"))
            dpsum = ctx.enter_context(
                tc.tile_pool(name="dpsum", bufs=2, space="PSUM"))
            rpsum = ctx.enter_context(
                tc.tile_pool(name="rpsum", bufs=1, space="PSUM"))

            bsm = cpool.tile([PB, W], bf16, name="bsm")
            nc.sync.dma_start(bsm[:], bsm_d.ap())
            negi5 = cpool.tile([PB, PB], bf16, name="negi5")
            nc.sync.dma_start(negi5[:], negi5_d.ap())
            ones = cpool.tile([PB, 1], bf16, name="ones")
            nc.vector.memset(ones[:], 1.0)
            acc = cpool.tile([PB, 24], f32, name="acc")
            # trigger Sigmoid/Abs ACT table loads during the first DMA
            dummy = cpool.tile([1, 2], bf16, name="dummy")
            nc.vector.memset(dummy[:], 0.0)
            nc.scalar.activation(dummy[:, 0:1], dummy[:, 1:2], Act.Sigmoid)
            nc.scalar.activation(dummy[:, 0:1], dummy[:, 1:2], Act.Abs)

            # persistent red PSUM tiles; slot s = 2*b + (0=pa, 1=tt) maps to
            # tile s//6, partition row 32*((s%6)//2), column half s%2.
            # Each sum is accumulated as 8 matmuls of 256 cols (4 k-blocks x
            # 2 column halves folded into the same range via PSUM accumulate).
            reds = [rpsum.tile([PB, W], f32, name=f"red{i}") for i in range(3)]
            redsb = cpool.tile([PB, 3, W], f32, name="redsb")

            def w_pass(b, mt3, ut3):
                """PE box-W matmuls + PSUM->SBUF scaled copies for image b."""
                for j in range(NB):
                    uj = upsum.tile([PB, W], f32, name="uj")
                    nc.tensor.matmul(uj[:], mt3[:, 1, 128 * j:128 * j + 128],
                                     bsm[:], start=True, stop=False,
                                     skip_group_check=True)
                    for k in (0, 2, 3):
                        s0, s1, c0, c1 = WIN[k]
                        nc.tensor.matmul(
                            uj[:, c0:c1], mt3[:, k, 128 * j:128 * j + 128],
                            bsm[:, s0:s1], start=False, stop=(k == 3),
                            skip_group_check=True)
                    # u' = (5/961) * U_j -> bf16 (2 copies on ACT, 2 on DVE
                    # per image for engine balance)
                    if j % 2 == 0:
                        nc.scalar.activation(ut3[:, j, :], uj[:], Act.Copy,
                                             bias=0.0, scale=5.0 / 961.0)
                    else:
                        nc.vector.tensor_scalar(ut3[:, j, :], uj[:],
                                                5.0 / 961.0, None, Alu.mult)

            def h_pass(mt3, ut3, at3):
                """PE box-H + ident matmuls, ACT abs -> a tiles."""
                for Wb in range(NB):
                    dw = dpsum.tile([PB, W], f32, name="dw")
                    nc.tensor.matmul(dw[:], negi5[:], mt3[:, Wb, :],
                                     start=True, stop=False,
                                     skip_group_check=True)
                    for j in range(NB):
                        s0, s1, c0, c1 = HWIN[j]
                        nc.tensor.matmul(
                            dw[:, c0:c1],
                            ut3[:, j, 128 * Wb:128 * Wb + 128],
                            bsm[:, s0:s1], start=False, stop=(j == 3),
                            skip_group_check=True)
                    nc.scalar.activation(at3[:, Wb, :], dw[:], Act.Abs,
                                         bias=0.0, scale=1.0)

            def products(b, mt, at, sg):
                s2 = scr.tile([PB, NB * W], bf16, name="s2", tag="s2")
                nc.vector.scalar_tensor_tensor(
                    s2[:], at[:], 1.0, mt[:], Alu.add, Alu.mult,
                    accum_out=acc[:, 2 * b:2 * b + 1])
                pa = scr.tile([PB, NB * W], bf16, name="pa", tag="pa")
                nc.vector.tensor_tensor(pa[:], sg[:], at[:], Alu.mult)
                tt = scr.tile([PB, NB * W], bf16, name="tt", tag="tt")
                nc.vector.tensor_tensor(tt[:], s2[:], sg[:], Alu.mult)
                return pa, tt

            def reductions(b, pa, tt):
                pa3 = pa.rearrange("p (k h) -> p k h", k=NB)
                tt3 = tt.rearrange("p (k h) -> p k h", k=NB)
                for t, pr3 in ((0, pa3), (1, tt3)):
                    s = 2 * b + t
                    tl, R, ch = s // 6, 32 * ((s % 6) // 2), 256 * (s % 2)
                    out = reds[tl][R:R + 1, ch:ch + 256]
                    for k in range(NB):
                        for hh in range(2):
                            nc.tensor.matmul(
                                out, ones[:],
                                pr3[:, k, 256 * hh:256 * hh + 256],
                                start=(k == 0 and hh == 0),
                                stop=(k == 3 and hh == 1),
                                skip_group_check=True)

            # 2-stage software pipeline.  Per round b each engine STARTS with
            # work whose inputs resolved last round:
            #   PE:  H(b-1) [u'(b-1) ready]  ->  W(b)  ->  red(b-2)
            #   ACT: abs(b-1) inline in H  ->  u-copies(b) inline in W ->
            #        sigmoid(b) last
            #   DVE: products(b-1) [after abs]  ->  u-copies(b) (odd b)
            # DMA prefetches image b+1 during round b.
            dmad = {}

            def prefetch(b):
                if b >= B_PER_CORE or b in dmad:
                    return
                mt = mpool.tile([PB, NB * W], bf16, name="mt")
                if b == 0:
                    # j-chunked first load: W(0, j) only needs columns
                    # [128j, 128j+128) of each k-block, so the first W
                    # matmuls can start after ~1/4 of the mask has landed.
                    mt4 = mt.rearrange("p (k h) -> p k h", k=NB)
                    src = maskT_d.ap()[b].rearrange("p (k h) -> p k h", k=NB)
                    for j in range(NB):
                        nc.sync.dma_start(mt4[:, :, 128 * j:128 * (j + 1)],
                                          src[:, :, 128 * j:128 * (j + 1)])
                else:
                    nc.sync.dma_start(mt[:], maskT_d.ap()[b])
                pt = ppool.tile([PB, NB * W], bf16, name="pt")
                nc.sync.dma_start(pt[:], predT_d.ap()[b])
                dmad[b] = [mt, pt]

            stash = {}
            prefetch(0)
            for b in range(B_PER_CORE + 2):
                prefetch(b + 1)

                if b >= 1 and b - 1 < B_PER_CORE:
                    mt_p, mt3_p, ut3_p, sg_p = stash[b - 1]
                    at = apool.tile([PB, NB * W], bf16, name="at")
                    at3 = at.rearrange("p (k h) -> p k h", k=NB)
                    h_pass(mt3_p, ut3_p, at3)
                    pa, tt = products(b - 1, mt_p, at, sg_p)
                    stash[b - 1] = [pa, tt]

                if b < B_PER_CORE:
                    mt, pt = dmad.pop(b)
                    mt3 = mt.rearrange("p (k h) -> p k h", k=NB)
                    ut = upool.tile([PB, NB * W], bf16, name="ut")
                    ut3 = ut.rearrange("p (j w) -> p j w", j=NB)
                    w_pass(b, mt3, ut3)

                if b >= 2:
                    pa, tt = stash.pop(b - 2)
                    reductions(b - 2, pa, tt)
                    # drain each red tile as soon as its last slot is done
                    # (slots 0-5 -> tile 0 after image 2, 6-11 -> tile 1
                    # after image 5, 12-15 -> tile 2 after image 7), so the
                    # epilogue only waits on the final tile.
                    done = b - 2
                    if done in (2, 5, 7):
                        i = min(done // 3, 2)
                        nc.scalar.copy(redsb[:, i, :], reds[i][:])
                        nc.sync.dma_start(red_d.ap()[:, i, :],
                                          redsb[:, i, :])

                if b < B_PER_CORE:
                    # sigmoid last on the ACT queue: sg(b) is not needed
                    # until round b+1.
                    sg = sgpool.tile([PB, NB * W], bf16, name="sg")
                    nc.scalar.activation(sg[:], pt[:], Act.Sigmoid,
                                         accum_out=acc[:, 16 + b:17 + b])
                    stash[b] = [mt, mt3, ut3, sg]

            # ---- epilogue (red tiles already drained in-loop) ----
            nc.sync.dma_start(acc_d.ap(), acc[:])

    nc.compile()
    return nc


_NC = None


def _get_nc():
    global _NC
    if _NC is None:
        _NC = _build()
    return _NC


def _in_maps(pred, mask):
    bsm, negi5 = _host_constants()
    pred = np.asarray(pred, dtype=np.float32).reshape(64, H, W)
    mask = np.asarray(mask, dtype=np.float32).reshape(64, H, W)
    ims = []
    for c in range(N_CORES):
        sl = slice(c * B_PER_CORE, (c + 1) * B_PER_CORE)
        # device layout [b, p, k*512 + h] = img[b, h, 128k + p]
        pr = pred[sl].reshape(B_PER_CORE, H, NB, PB).transpose(0, 3, 2, 1)
        mr = mask[sl].reshape(B_PER_CORE, H, NB, PB).transpose(0, 3, 2, 1)
        ims.append({
            "predT": np.ascontiguousarray(
                pr.reshape(B_PER_CORE, PB, NB * W)).astype(ml_dtypes.bfloat16),
            "maskT": np.ascontiguousarray(
                mr.reshape(B_PER_CORE, PB, NB * W)).astype(ml_dtypes.bfloat16),
            "bsm": bsm,
            "negi5": negi5,
        })
    return ims


def _host_reduce(results):
    wd = []
    for r in results:
        a = r["acc"].astype(np.float64)
        rd = r["red"].astype(np.float64)
        for b in range(B_PER_CORE):
            s2sum = a[:, 2 * b].sum()            # sum(mask*weit)
            psum = a[:, 16 + b].sum()            # sum(p)

            def slot(t):
                s = 2 * b + t
                tl, R, ch = s // 6, 32 * ((s % 6) // 2), 256 * (s % 2)
                return rd[R, tl, ch:ch + 256].sum()

            pasum = slot(0)                # sum(p*(weit-1))
            ttsum = slot(1)                # sum(p*mask*weit)
            union = psum + pasum + s2sum
            inter = ttsum
            wd.append(1.0 - (2.0 * inter + 0.5) / (union + 0.5))
    return np.array(np.mean(wd), dtype=np.float32)


def kernel(pred, mask):
    nc = _get_nc()
    res = run_bass_kernel_spmd(nc, _in_maps(pred, mask),
                               core_ids=list(range(N_CORES)))
    return _host_reduce(res.results)
